# revision 17
# baseline (speedup 1.0000x reference)
"""GSphereNet message-passing layer on 8 TRN2 NeuronCores (Bass/Tile).

Math: out = x + relu((segsum(feat97) @ W_aug) @ W1 + b1) @ W2 + b2
where feat97 = [rbf|angle|1] per edge -- the edge projection commutes with
segment_sum, so aggregation moves 97 floats/edge, and W_aug @ W1 folds into
a single [97,512] matrix on host (the device MLP is two GEMMs).

Distribution: edges are routed BY DESTINATION SHARD on host -- core c gets
exactly the edges targeting its 6272-node slice, so there is no collective
at all. Per 128-node chunk, segment-sum runs as a one-hot matmul on the
TensorEngine: aggT[feat, node] = sum_tiles F_tile[128e,104f].T @ S[128e,128n]
with S = (iota == dest) built by one DVE compare per tile. bf16 operands,
fp32 PSUM accumulation (measured emulation rel-err ~1.8e-3 vs 2e-2 gate).
"""
import sys

sys.path.insert(0, '/opt/trn_rl_repo')

import os
import numpy as np
from ml_dtypes import bfloat16

P = 128
N_NODES = 50000
N_EDGES = 400000
EMBED = 512
RBF = 64
ANG = 32
N_CORES = 8

NODES_PAD = 50176              # 8 * 6272
RPC = NODES_PAD // N_CORES     # 6272 rows per core
NCH = RPC // P                 # 49 chunks of 128 nodes per core
FEAT = 104                     # 97 used features padded to 8-elem alignment
NCHUNK = 512                   # node-MLP group (4 chunks)
GT = 64                        # token tiles per DMA group


def _host_pack(rows, rbf_feature, angle_feature):
    """Route edges by destination shard; per 128-node chunk pad to whole
    128-edge tiles with a tile schedule shared by all cores (SPMD).
    Returns (tok_list, dst_list, ntiles)."""
    order = np.argsort(rows, kind='stable')
    rs = rows[order]
    gstart = np.searchsorted(rs, np.arange(0, NODES_PAD, P))  # 392 chunk starts
    gcnt = np.diff(np.r_[gstart, N_EDGES])
    counts = gcnt.reshape(N_CORES, NCH)                       # [core, chunk]
    tiles_per_chunk = np.maximum(1, -(-counts.max(axis=0) // P))  # [NCH]
    offs = np.concatenate([[0], np.cumsum(tiles_per_chunk)])
    ntiles = int(offs[-1])

    # position of each (sorted) edge inside its core's padded stream
    rank = np.arange(N_EDGES) - np.repeat(gstart, gcnt)
    gid = rs // P                      # global chunk id 0..391
    chunk = gid % NCH
    core_s = gid // NCH
    pos = offs[chunk] * P + rank       # stream slot within core

    feat = np.zeros((N_EDGES, FEAT), dtype=np.float32)
    feat[:, :RBF] = rbf_feature
    feat[:, RBF:RBF + ANG] = angle_feature
    feat[:, 96] = 1.0
    featb = feat.astype(bfloat16)

    dloc = (rs % P).astype(np.float32)

    tok_list, dst_list = [], []
    for c in range(N_CORES):
        m = core_s == c
        stream = np.zeros((ntiles * P, FEAT), dtype=bfloat16)
        dstr = np.zeros((ntiles * P,), dtype=np.float16)
        stream[pos[m]] = featb[order[m]]
        dstr[pos[m]] = dloc[m]
        tok_list.append(np.ascontiguousarray(
            stream.reshape(ntiles, P, FEAT).transpose(1, 0, 2)))
        dst_list.append(np.ascontiguousarray(dstr.reshape(ntiles, P).T))
    return tok_list, dst_list, ntiles, tiles_per_chunk


def _build_program(ntiles, tiles_per_chunk, mybir, bacc, tile):
    f32 = mybir.dt.float32
    f16 = mybir.dt.float16
    bf16 = mybir.dt.bfloat16
    nc = bacc.Bacc("TRN2", target_bir_lowering=False, debug=False,
                   num_devices=N_CORES)
    tok_d = nc.dram_tensor("tokens", [P, ntiles, FEAT], bf16, kind="ExternalInput")
    dst_d = nc.dram_tensor("dests", [P, ntiles], f16, kind="ExternalInput")
    xb_d = nc.dram_tensor("xb", [P, NCH, EMBED], bf16, kind="ExternalInput")
    wc_d = nc.dram_tensor("wc", [FEAT, EMBED], bf16, kind="ExternalInput")
    w2_d = nc.dram_tensor("w2d", [P, 4, EMBED], bf16, kind="ExternalInput")
    b1_d = nc.dram_tensor("b1t", [P, 4], f32, kind="ExternalInput")
    out_d = nc.dram_tensor("out_s", [P, NCH, EMBED], f32, kind="ExternalOutput")
    iota_d = nc.inline_tensor(
        np.tile(np.arange(P, dtype=np.float16), (P, 1)), "iota")
    ident_d = nc.inline_tensor(np.eye(P, dtype=np.float32).astype(bfloat16),
                               "ident")

    offs = np.concatenate([[0], np.cumsum(tiles_per_chunk)])
    act_relu = mybir.ActivationFunctionType.Relu
    is_eq = mybir.AluOpType.is_equal
    add_op = mybir.AluOpType.add

    with tile.TileContext(nc) as tc:
        with (
            tc.tile_pool(name="wts", bufs=1) as wpool,
            tc.tile_pool(name="toks", bufs=3) as fpool,
            tc.tile_pool(name="sones", bufs=4) as spool,
            tc.tile_pool(name="aggt", bufs=1) as apool,
            tc.tile_pool(name="hts", bufs=2) as hpool,
            tc.tile_pool(name="xin", bufs=2) as xpool,
            tc.tile_pool(name="oout", bufs=2) as opool,
            tc.tile_pool(name="psa", bufs=2, space="PSUM") as psa,
            tc.tile_pool(name="psh", bufs=2, space="PSUM") as psh,
            tc.tile_pool(name="pso", bufs=2, space="PSUM") as pso,
        ):
            iota = wpool.tile([P, P], f16)
            nc.sync.dma_start(out=iota[:], in_=iota_d[:])
            wc = wpool.tile([FEAT, EMBED], bf16)
            nc.sync.dma_start(out=wc[:], in_=wc_d[:])
            w2 = wpool.tile([P, 4, EMBED], bf16)
            nc.sync.dma_start(out=w2[:], in_=w2_d[:])
            b1t = wpool.tile([P, 4], f32)
            nc.sync.dma_start(out=b1t[:], in_=b1_d[:])
            dst = wpool.tile([P, ntiles], f16)
            nc.sync.dma_start(out=dst[:], in_=dst_d[:])
            ident = wpool.tile([P, P], bf16)
            nc.sync.dma_start(out=ident[:], in_=ident_d[:])
            aggT = apool.tile([FEAT, RPC], bf16)
            maxt = int(tiles_per_chunk.max())

            cur = {}

            def tok_ap(t):
                g = t // GT
                if cur.get('g') != g:
                    gsz = min(GT, ntiles - g * GT)
                    ft = fpool.tile([P, GT, FEAT], bf16, tag="fg")
                    nc.sync.dma_start(out=ft[:, :gsz, :],
                                      in_=tok_d[:, g * GT:g * GT + gsz, :])
                    cur['g'] = g
                    cur['t'] = ft
                return cur['t'][:, t - cur['g'] * GT, :]

            n_groups = -(-RPC // NCHUNK)
            for gi in range(n_groups):
                n0 = gi * NCHUNK
                nw = min(NCHUNK, RPC - n0)
                # ---- aggregate the 4 chunks of this node group ----
                for ci in range(n0 // P, (n0 + nw) // P):
                    ps = psa.tile([FEAT, P], f32, tag="agg")
                    t0, t1 = int(offs[ci]), int(offs[ci + 1])
                    nt = t1 - t0
                    S3 = spool.tile([P, maxt, P], f16, tag="S")
                    nc.vector.tensor_tensor(
                        S3[:, :nt, :],
                        iota[:].unsqueeze(1).broadcast_to([P, nt, P]),
                        dst[:, t0:t1].unsqueeze(2).broadcast_to([P, nt, P]),
                        is_eq)
                    for t in range(t0, t1):
                        fa = tok_ap(t)
                        nc.tensor.matmul(ps[:], fa, S3[:, t - t0, :],
                                         start=(t == t0), stop=(t == t1 - 1))
                    nc.scalar.copy(out=aggT[:, ci * P:(ci + 1) * P], in_=ps[:])
                # ---- node MLP on this group ----
                hT = hpool.tile([P, 4, NCHUNK], bf16, tag="hT")
                for f in range(4):
                    ph = psh.tile([P, NCHUNK], f32, tag="h")
                    nc.tensor.matmul(ph[:, :nw], wc[:, f * P:(f + 1) * P],
                                     aggT[:, n0:n0 + nw], start=True, stop=True)
                    nc.scalar.activation(out=hT[:, f, :nw], in_=ph[:, :nw],
                                         func=act_relu, bias=b1t[:, f:f + 1])
                c0 = n0 // P
                xt = xpool.tile([P, NCHUNK // P, EMBED], bf16, tag="x")
                nc.sync.dma_start(
                    out=xt[:, :nw // P, :],
                    in_=xb_d[:, c0:c0 + nw // P, :])
                ot = opool.tile([P, NCHUNK // P, EMBED], f32, tag="o")
                for t in range(nw // P):
                    po = pso.tile([P, EMBED], f32, tag="po")
                    nc.tensor.matmul(po[:], ident[:], xt[:, t, :],
                                     start=True, stop=False)
                    for k in range(4):
                        nc.tensor.matmul(po[:], hT[:, k, t * P:(t + 1) * P],
                                         w2[:, k, :],
                                         start=False, stop=(k == 3))
                    ceng = nc.vector if t % 2 == 0 else nc.scalar
                    if ceng is nc.vector:
                        nc.vector.tensor_copy(ot[:, t, :], po[:])
                    else:
                        nc.scalar.copy(out=ot[:, t, :], in_=po[:])
                nc.sync.dma_start(
                    out=out_d[:, c0:c0 + nw // P, :],
                    in_=ot[:, :nw // P, :])
    nc.finalize()
    return nc


def kernel(x, edge_index, rbf_feature, angle_feature, W_edge, b_edge, W1, b1, W2, b2):
    from concourse import bacc, tile, mybir
    from concourse.bass_utils import run_bass_kernel_spmd

    x = np.asarray(x, dtype=np.float32)
    rows = np.asarray(edge_index[0], dtype=np.int64)
    tok_list, dst_list, ntiles, tiles_per_chunk = _host_pack(
        rows, np.asarray(rbf_feature, np.float32),
        np.asarray(angle_feature, np.float32))

    # fold W_aug @ W1 on host (fp64), plus x+b2
    W_aug = np.zeros((FEAT, EMBED), dtype=np.float64)
    W_aug[:RBF + ANG] = np.asarray(W_edge, np.float64)
    W_aug[96] = np.asarray(b_edge, np.float64)
    wc = (W_aug @ np.asarray(W1, np.float64)).astype(np.float32).astype(bfloat16)
    w2d = np.ascontiguousarray(
        np.asarray(W2, np.float32).reshape(4, P, EMBED).transpose(1, 0, 2)
    ).astype(bfloat16)
    b1t = np.ascontiguousarray(np.asarray(b1, np.float32).reshape(4, P).T)
    xb_full = np.zeros((NODES_PAD, EMBED), dtype=bfloat16)
    xb_full[:N_NODES] = (x + np.asarray(b2, np.float32)).astype(bfloat16)

    in_maps = []
    for c in range(N_CORES):
        xbs = xb_full[c * RPC:(c + 1) * RPC]
        in_maps.append({
            "tokens": tok_list[c], "dests": dst_list[c],
            "xb": np.ascontiguousarray(
                xbs.reshape(NCH, P, EMBED).transpose(1, 0, 2)),
            "wc": wc, "w2d": w2d, "b1t": b1t,
        })

    nc = _build_program(ntiles, tiles_per_chunk, mybir, bacc, tile)
    if os.environ.get('DEBUG_SIM') == '1':
        from concourse import bass_interp
        results = []
        for c in range(N_CORES):
            sim = bass_interp.CoreSim(nc)
            for k, v in in_maps[c].items():
                sim.tensor(k)[:] = v
            sim.tensor('out_s')[:] = 0
            sim.simulate()
            results.append({'out_s': np.array(sim.tensor('out_s'))})

        class R:
            pass
        res = R()
        res.results = results
        res.exec_time_ns = None
        res.instructions_and_trace = None
        res.profile_json = None
    else:
        res = run_bass_kernel_spmd(nc, in_maps, list(range(N_CORES)))
    global LAST_RESULT
    LAST_RESULT = res
    out = np.concatenate(
        [res.results[c]["out_s"].transpose(1, 0, 2).reshape(RPC, EMBED)
         for c in range(N_CORES)], axis=0)
    return out[:N_NODES]


LAST_RESULT = None


# revision 23
# speedup vs baseline: 1.0221x; 1.0221x over previous
"""GSphereNet message-passing layer on 8 TRN2 NeuronCores (Bass/Tile).

Math: out = x + relu((segsum(feat97) @ W_aug) @ W1 + b1) @ W2 + b2
where feat97 = [rbf|angle|1] per edge -- the edge projection commutes with
segment_sum, so aggregation moves 97 floats/edge, and W_aug @ W1 folds into
a single [97,512] matrix on host (the device MLP is two GEMMs).

Distribution: edges are routed BY DESTINATION SHARD on host -- core c gets
exactly the edges targeting its 6272-node slice, so there is no collective
at all. Per 128-node chunk, segment-sum runs as a one-hot matmul on the
TensorEngine: aggT[feat, node] = sum_tiles F_tile[128e,104f].T @ S[128e,128n]
with S = (iota == dest) built by one DVE compare per tile. bf16 operands,
fp32 PSUM accumulation (measured emulation rel-err ~1.8e-3 vs 2e-2 gate).
"""
import sys

sys.path.insert(0, '/opt/trn_rl_repo')

import os
import numpy as np
from ml_dtypes import bfloat16

P = 128
N_NODES = 50000
N_EDGES = 400000
EMBED = 512
RBF = 64
ANG = 32
N_CORES = 8

NODES_PAD = 50176              # 8 * 6272
RPC = NODES_PAD // N_CORES     # 6272 rows per core
NCH = RPC // P                 # 49 chunks of 128 nodes per core
FEAT = 104                     # 97 used features padded to 8-elem alignment
NCHUNK = 512                   # node-MLP group (4 chunks)
GT = 64                        # token tiles per DMA group


def _host_pack(rows, rbf_feature, angle_feature):
    """Route edges by destination shard; per 128-node chunk pad to whole
    128-edge tiles with a tile schedule shared by all cores (SPMD).
    Returns (tok_list, dst_list, ntiles)."""
    order = np.argsort(rows, kind='stable')
    rs = rows[order]
    gstart = np.searchsorted(rs, np.arange(0, NODES_PAD, P))  # 392 chunk starts
    gcnt = np.diff(np.r_[gstart, N_EDGES])
    counts = gcnt.reshape(N_CORES, NCH)                       # [core, chunk]
    tiles_per_chunk = np.maximum(1, -(-counts.max(axis=0) // P))  # [NCH]
    offs = np.concatenate([[0], np.cumsum(tiles_per_chunk)])
    ntiles = int(offs[-1])

    # position of each (sorted) edge inside its core's padded stream
    rank = np.arange(N_EDGES) - np.repeat(gstart, gcnt)
    gid = rs // P                      # global chunk id 0..391
    chunk = gid % NCH
    core_s = gid // NCH
    pos = offs[chunk] * P + rank       # stream slot within core

    feat = np.zeros((N_EDGES, FEAT), dtype=np.float32)
    feat[:, :RBF] = rbf_feature
    feat[:, RBF:RBF + ANG] = angle_feature
    feat[:, 96] = 1.0
    featb = feat.astype(bfloat16)

    dloc = (rs % P).astype(np.float32)

    tok_list, dst_list = [], []
    for c in range(N_CORES):
        m = core_s == c
        stream = np.zeros((ntiles * P, FEAT), dtype=bfloat16)
        dstr = np.zeros((ntiles * P,), dtype=bfloat16)
        stream[pos[m]] = featb[order[m]]
        dstr[pos[m]] = dloc[m]
        tok_list.append(np.ascontiguousarray(
            stream.reshape(ntiles, P, FEAT).transpose(1, 0, 2)))
        dst_list.append(np.ascontiguousarray(dstr.reshape(ntiles, P).T))
    return tok_list, dst_list, ntiles, tiles_per_chunk


def _build_program(ntiles, tiles_per_chunk, mybir, bacc, tile):
    f32 = mybir.dt.float32
    f16 = mybir.dt.float16
    bf16 = mybir.dt.bfloat16
    nc = bacc.Bacc("TRN2", target_bir_lowering=False, debug=False,
                   num_devices=N_CORES)
    tok_d = nc.dram_tensor("tokens", [P, ntiles, FEAT], bf16, kind="ExternalInput")
    dst_d = nc.dram_tensor("dests", [P, ntiles], bf16, kind="ExternalInput")
    xb_d = nc.dram_tensor("xb", [P, NCH, EMBED], bf16, kind="ExternalInput")
    wc_d = nc.dram_tensor("wc", [FEAT, EMBED], bf16, kind="ExternalInput")
    w2_d = nc.dram_tensor("w2d", [P, 4, EMBED], bf16, kind="ExternalInput")
    b1_d = nc.dram_tensor("b1t", [P, 4], f32, kind="ExternalInput")
    out_d = nc.dram_tensor("out_s", [P, NCH, EMBED], bf16, kind="ExternalOutput")
    iota_d = nc.inline_tensor(
        np.tile(np.arange(P, dtype=np.float32), (P, 1)).astype(bfloat16), "iota")
    ident_d = nc.inline_tensor(np.eye(P, dtype=np.float32).astype(bfloat16),
                               "ident")

    offs = np.concatenate([[0], np.cumsum(tiles_per_chunk)])
    act_relu = mybir.ActivationFunctionType.Relu
    is_eq = mybir.AluOpType.is_equal
    add_op = mybir.AluOpType.add

    with tile.TileContext(nc) as tc:
        with (
            tc.tile_pool(name="wts", bufs=1) as wpool,
            tc.tile_pool(name="toks", bufs=3) as fpool,
            tc.tile_pool(name="sones", bufs=4) as spool,
            tc.tile_pool(name="aggt", bufs=1) as apool,
            tc.tile_pool(name="hts", bufs=2) as hpool,
            tc.tile_pool(name="xin", bufs=2) as xpool,
            tc.tile_pool(name="oout", bufs=2) as opool,
            tc.tile_pool(name="psa", bufs=2, space="PSUM") as psa,
            tc.tile_pool(name="psh", bufs=2, space="PSUM") as psh,
            tc.tile_pool(name="pso", bufs=2, space="PSUM") as pso,
        ):
            # token-group boundaries: small first group so the PE can start
            # quickly, then full-size groups
            g_bounds = [0, min(16, ntiles)]
            while g_bounds[-1] < ntiles:
                g_bounds.append(min(g_bounds[-1] + GT, ntiles))
            cur = {}

            def tok_ap(t):
                g = cur.get('g')
                if g is None or not (g_bounds[g] <= t < g_bounds[g + 1]):
                    g = next(i for i in range(len(g_bounds) - 1)
                             if g_bounds[i] <= t < g_bounds[i + 1])
                    gsz = g_bounds[g + 1] - g_bounds[g]
                    ft = fpool.tile([P, GT, FEAT], bf16, tag="fg")
                    nc.sync.dma_start(
                        out=ft[:, :gsz, :],
                        in_=tok_d[:, g_bounds[g]:g_bounds[g + 1], :])
                    cur['g'] = g
                    cur['t'] = ft
                return cur['t'][:, t - g_bounds[cur['g']], :]

            tok_ap(0)  # queue the first token load before anything else
            dst = wpool.tile([P, ntiles], bf16)
            nc.sync.dma_start(out=dst[:], in_=dst_d[:])
            iota = wpool.tile([P, P], bf16)
            nc.sync.dma_start(out=iota[:], in_=iota_d[:])
            wc = wpool.tile([FEAT, EMBED], bf16)
            nc.sync.dma_start(out=wc[:], in_=wc_d[:])
            w2 = wpool.tile([P, 4, EMBED], bf16)
            nc.sync.dma_start(out=w2[:], in_=w2_d[:])
            b1t = wpool.tile([P, 4], f32)
            nc.sync.dma_start(out=b1t[:], in_=b1_d[:])
            ident = wpool.tile([P, P], bf16)
            nc.sync.dma_start(out=ident[:], in_=ident_d[:])
            aggT = apool.tile([FEAT, RPC], bf16)
            maxt = int(tiles_per_chunk.max())

            n_groups = -(-RPC // NCHUNK)
            for gi in range(n_groups):
                n0 = gi * NCHUNK
                nw = min(NCHUNK, RPC - n0)
                # ---- aggregate the 4 chunks of this node group ----
                ps4 = psa.tile([FEAT, NCHUNK // P, P], f32, tag="agg")
                for ci in range(n0 // P, (n0 + nw) // P):
                    si = ci - n0 // P
                    t0, t1 = int(offs[ci]), int(offs[ci + 1])
                    nt = t1 - t0
                    S3 = spool.tile([P, maxt, P], bf16, tag="S")
                    nc.vector.tensor_tensor(
                        S3[:, :nt, :],
                        iota[:].unsqueeze(1).broadcast_to([P, nt, P]),
                        dst[:, t0:t1].unsqueeze(2).broadcast_to([P, nt, P]),
                        is_eq)
                    for t in range(t0, t1):
                        fa = tok_ap(t)
                        nc.tensor.matmul(ps4[:, si, :], fa, S3[:, t - t0, :],
                                         start=(t == t0), stop=(t == t1 - 1))
                nc.scalar.copy(out=aggT[:, n0:n0 + nw],
                               in_=ps4[:, :nw // P, :])
                # ---- node MLP on this group ----
                hT = hpool.tile([P, 4, NCHUNK], bf16, tag="hT")
                for f in range(4):
                    ph = psh.tile([P, NCHUNK], f32, tag="h")
                    nc.tensor.matmul(ph[:, :nw], wc[:, f * P:(f + 1) * P],
                                     aggT[:, n0:n0 + nw], start=True, stop=True)
                    nc.scalar.activation(out=hT[:, f, :nw], in_=ph[:, :nw],
                                         func=act_relu, bias=b1t[:, f:f + 1])
                c0 = n0 // P
                xt = xpool.tile([P, NCHUNK // P, EMBED], bf16, tag="x")
                nc.sync.dma_start(
                    out=xt[:, :nw // P, :],
                    in_=xb_d[:, c0:c0 + nw // P, :])
                ot = opool.tile([P, NCHUNK // P, EMBED], bf16, tag="o")
                for t in range(nw // P):
                    po = pso.tile([P, EMBED], f32, tag="po")
                    nc.tensor.matmul(po[:], ident[:], xt[:, t, :],
                                     start=True, stop=False)
                    for k in range(4):
                        nc.tensor.matmul(po[:], hT[:, k, t * P:(t + 1) * P],
                                         w2[:, k, :],
                                         start=False, stop=(k == 3))
                    ceng = nc.vector if t % 2 == 0 else nc.scalar
                    if ceng is nc.vector:
                        nc.vector.tensor_copy(ot[:, t, :], po[:])
                    else:
                        nc.scalar.copy(out=ot[:, t, :], in_=po[:])
                nc.sync.dma_start(
                    out=out_d[:, c0:c0 + nw // P, :],
                    in_=ot[:, :nw // P, :])
    nc.finalize()
    return nc


def kernel(x, edge_index, rbf_feature, angle_feature, W_edge, b_edge, W1, b1, W2, b2):
    from concourse import bacc, tile, mybir
    from concourse.bass_utils import run_bass_kernel_spmd

    x = np.asarray(x, dtype=np.float32)
    rows = np.asarray(edge_index[0], dtype=np.int64)
    tok_list, dst_list, ntiles, tiles_per_chunk = _host_pack(
        rows, np.asarray(rbf_feature, np.float32),
        np.asarray(angle_feature, np.float32))

    # fold W_aug @ W1 on host (fp64), plus x+b2
    W_aug = np.zeros((FEAT, EMBED), dtype=np.float64)
    W_aug[:RBF + ANG] = np.asarray(W_edge, np.float64)
    W_aug[96] = np.asarray(b_edge, np.float64)
    wc = (W_aug @ np.asarray(W1, np.float64)).astype(np.float32).astype(bfloat16)
    w2d = np.ascontiguousarray(
        np.asarray(W2, np.float32).reshape(4, P, EMBED).transpose(1, 0, 2)
    ).astype(bfloat16)
    b1t = np.ascontiguousarray(np.asarray(b1, np.float32).reshape(4, P).T)
    xb_full = np.zeros((NODES_PAD, EMBED), dtype=bfloat16)
    xb_full[:N_NODES] = (x + np.asarray(b2, np.float32)).astype(bfloat16)

    in_maps = []
    for c in range(N_CORES):
        xbs = xb_full[c * RPC:(c + 1) * RPC]
        in_maps.append({
            "tokens": tok_list[c], "dests": dst_list[c],
            "xb": np.ascontiguousarray(
                xbs.reshape(NCH, P, EMBED).transpose(1, 0, 2)),
            "wc": wc, "w2d": w2d, "b1t": b1t,
        })

    nc = _build_program(ntiles, tiles_per_chunk, mybir, bacc, tile)
    if os.environ.get('DEBUG_SIM') == '1':
        from concourse import bass_interp
        results = []
        for c in range(N_CORES):
            sim = bass_interp.CoreSim(nc)
            for k, v in in_maps[c].items():
                sim.tensor(k)[:] = v
            sim.tensor('out_s')[:] = 0
            sim.simulate()
            results.append({'out_s': np.array(sim.tensor('out_s'))})

        class R:
            pass
        res = R()
        res.results = results
        res.exec_time_ns = None
        res.instructions_and_trace = None
        res.profile_json = None
    else:
        res = run_bass_kernel_spmd(nc, in_maps, list(range(N_CORES)))
    global LAST_RESULT
    LAST_RESULT = res
    out = np.concatenate(
        [res.results[c]["out_s"].astype(np.float32)
         .transpose(1, 0, 2).reshape(RPC, EMBED)
         for c in range(N_CORES)], axis=0)
    return out[:N_NODES]


LAST_RESULT = None
